# revision 20
# baseline (speedup 1.0000x reference)
"""Self-contained Bass/Tile TRN2 kernel: 1-layer LSTM encoder over T=20 steps,
batch 65536, hidden 64, data-parallel over batch across 8 NeuronCores.

kernel(**inputs) takes the FULL unsharded inputs (obs_traj [20,65536,2] f32 +
small LSTM/Linear weights) and returns final_h [1, 65536, 64] f32.

Method (per core, batch shard of 8192, batch-major layout):
  - Embedding folded into the LSTM input projection on host:
      W_x = W_ih @ W_emb,  bias = b_ih + b_hh + W_ih @ b_emb.
  - ONE K=67 matmul per 128-batch group per step:
      out[128 batch, 256 gates] = lhsT.T @ rhs with lhsT = Xh[0:67, group]
      (rows 0:64 = h features, row 64 = const 1, rows 65:67 = x_t) and
      rhs = Wg[0:67, 256].  The const/x rows ride the h-return DMA-xbar
      transpose: the batch-major hb tile allocates 128 cols per group
      ([h(64) | 1 | x(2) | pad(61)]), so each 128x128 xbar block transposes
      straight into the K=67 operand.  x_t arrives batch-major from DRAM
      (one small DMA per step); no feature-major x staging, no second
      matmul, half the LDWEIGHTS.
  - Gate columns permuted to [i, f, o, g] on host.  All-tanh trick: sigma
    gates' W columns pre-scaled by 0.5 so one ACT Tanh over a full
    [128, 2048] PSUM view (2 groups per bank, 8 groups per super) gives
    tanh(raw/2) for i,f,o and tanh(raw) for g; sigma recovered with a
    single fused DVE tensor_scalar (add 1, mult 0.5) over the contiguous
    [i,f,o] slice.
  - Cell update: v = sig_i*tanh_g on GPSIMD; u = sig_f*c, c = u+v,
    h = sig_o*tanh(c) on DVE; tanh(c) on ACT batched 4 supers per
    instruction (FD2048); h-return transposes batched 4 supers per
    dma_start_transpose (2 per step).
"""

import numpy as np
import ml_dtypes

import concourse.bass as bass
import concourse.mybir as mybir
import concourse.tile as tile_mod
from concourse.tile import TileContext
import bass_rust as _bass_rust
from bass_rust import ScopedClock, VectorClock
from concourse.tile_scheduler import N_PROCS

BF16 = mybir.dt.bfloat16
F32 = mybir.dt.float32
AluOp = mybir.AluOpType

T_STEPS = 20
B_FULL = 65536
N_CORES = 8
BC = B_FULL // N_CORES          # 8192
HID = 64
NGATE = 256
GROUP = 128
NGROUPS = BC // GROUP           # 64
SUPER = 8                       # groups per super (8 x 256 gates = 4 banks)
NSUPER = NGROUPS // SUPER       # 8
SCOLS = SUPER * GROUP           # 1024
SPH = 4                         # supers per tanh(c) instruction / transpose
KROWS = 67                      # 64 h + 1 const + 2 x
HBK = 128                       # hb cols per group: [h(64) | 1 | x(2) | pad]


def _patched_drain_and_barrier(self, tick_clock, wait_clock):
    # This walrus build accepts at most ONE sync-wait per instruction; the
    # stock tail Drain carries one wait per live proc.  Emit one NOP per
    # proc instead, each carrying a single wait.
    gc = tick_clock.global_clock
    for p in range(N_PROCS):
        t = gc[p]
        if t <= 0:
            continue
        nop = self.nc.sync.nop(nofuse=True, hint=f"tail_wait_p{p}")
        wait_clock.add_sem_waits(
            nop.ins,
            ScopedClock(
                {None: VectorClock([t if q == p else 0 for q in range(N_PROCS)])}
            ),
        )
    self.nc.sync.drain()
    self.nc.all_engine_barrier()
    assert self.sems is not None
    popped = self.nc._tile_sem_poison_stack.pop()
    assert popped is self._sem_poison
    self.nc.clear_and_free_semaphores(list(self.sems.allocated().values()))
    self.nc.all_engine_barrier()


tile_mod.TileContext._drain_and_barrier = _patched_drain_and_barrier


def split_excess_waits(nc, max_waits=1):
    """Hoist excess semaphore waits onto same-engine NOPs placed directly
    before the instruction (the engine blocks at the same point)."""
    ctr = 0
    for fn in nc.m.functions:
        for bb in fn.blocks:
            il = bb.instructions
            i = 0
            while i < len(il):
                inst = il[i]
                si = inst.sync_info
                waits = list(si.on_wait) if si is not None and si.on_wait else []
                if len(waits) > max_waits:
                    sem_waits = [w for w in waits if w.sync_type == "semaphore"]
                    other = [w for w in waits if w.sync_type != "semaphore"]
                    keep_n = max(0, max_waits - len(other))
                    keep = other + sem_waits[:keep_n]
                    extra = sem_waits[keep_n:]
                    pos = i
                    for j in range(0, len(extra), max(1, max_waits)):
                        chunk = extra[j:j + max(1, max_waits)]
                        nop = mybir.InstNoOp(name=f"wsplit-{ctr}", ins=[],
                                             outs=[])
                        ctr += 1
                        nop.engine = inst.engine
                        nop.sync_info = _bass_rust.SyncInfo(
                            on_wait=chunk, on_update=[])
                        il.insert(pos, nop)
                        pos += 1
                        i += 1
                    inst.sync_info = _bass_rust.SyncInfo(
                        on_wait=keep,
                        on_update=list(si.on_update) if si.on_update else [])
                i += 1
    return ctr


def host_weights(W_emb, b_emb, W_ih, W_hh, b_ih, b_hh):
    W_x = W_ih @ W_emb                      # [256, 2]
    bias = b_ih + b_hh + W_ih @ b_emb       # [256]
    # permute gate columns from [i, f, g, o] to [i, f, o, g]
    perm = np.concatenate(
        [np.arange(0, 128), np.arange(192, 256), np.arange(128, 192)])
    colscale = np.concatenate(
        [np.full(192, 0.5), np.full(64, 1.0)]).astype(np.float32)
    Wc = np.zeros((KROWS, NGATE), np.float32)
    Wc[0:64] = W_hh.T[:, perm] * colscale
    Wc[64] = bias[perm] * colscale          # const-1 row
    Wc[65:67] = W_x.T[:, perm] * colscale
    return Wc.astype(ml_dtypes.bfloat16)


def build_nc(tr_supers=2):
    # tr_supers: supers per dma_start_transpose instruction (1, 2, or 4)
    nc = bass.Bass("TRN2", target_bir_lowering=False)
    xt = nc.dram_tensor("xt", [T_STEPS, 3, BC], BF16, kind="ExternalInput")
    wg = nc.dram_tensor("wg", [KROWS, NGATE], BF16, kind="ExternalInput")
    hout = nc.dram_tensor("hout", [BC, HID], F32, kind="ExternalOutput")

    with TileContext(nc) as tc:
        with (
            tc.tile_pool(name="state", bufs=1) as state_pool,
            tc.tile_pool(name="tpool", bufs=10) as t_pool,
            tc.tile_pool(name="uv", bufs=8) as uv_pool,
            tc.tile_pool(name="tcpool", bufs=4) as tc_pool,
            tc.tile_pool(name="hf", bufs=8) as hf_pool,
            tc.tile_pool(name="psum", bufs=2, space="PSUM") as psum_pool,
        ):
            Wg = state_pool.tile([128, NGATE], BF16, tag="Wg")
            nc.sync.dma_start(Wg[0:KROWS, :], wg[:])

            Xh2a = state_pool.tile([128, BC], BF16, tag="Xh2a")
            Xh2b = state_pool.tile([128, BC], BF16, tag="Xh2b")
            hbA = state_pool.tile([128, NGROUPS * HBK], BF16, tag="hbA")
            hbB = state_pool.tile([128, NGROUPS * HBK], BF16, tag="hbB")
            C = state_pool.tile([128, NGROUPS * HID], BF16, tag="C")

            xstA = state_pool.tile([16, BC], BF16, tag="xstA")
            xstB = state_pool.tile([16, BC], BF16, tag="xstB")

            Xh2_of = [Xh2a, Xh2b]
            hb_of = [hbA, hbB]
            xst_of = [xstA, xstB]
            nc.vector.memset(Xh2a[:, :], 0.0)
            nc.sync.dma_start(Xh2a[64:67, :], xt[0])
            # hb pad columns / xst junk rows are never written; memset once
            # so the transposes read defined data (lands in unused Xh2 rows)
            nc.vector.memset(hbA[:, :], 0.0)
            nc.gpsimd.memset(hbB[:, :], 0.0)
            nc.vector.memset(xstA[:, :], 0.0)
            nc.vector.memset(xstB[:, :], 0.0)

            Tts = {}            # (t, s) -> T tile
            tcvs = {}           # (t, chunk_lo) -> tanh(c) tile

            def emit_tanh_c(t, lo):
                tcols = SPH * SUPER * HID
                tcv = tc_pool.tile([128, tcols], BF16, tag="tc")
                nc.scalar.activation(
                    tcv[:],
                    C[:, lo * SUPER * HID:lo * SUPER * HID + tcols],
                    mybir.ActivationFunctionType.Tanh)
                tcvs[(t, lo)] = tcv

            def emit_hb_tr(t, lo):
                # h = sig_o * tanh(c) and the h-return transposes for
                # supers lo..lo+SPH-1 of step t.  The transposed 128-col
                # blocks carry h plus the const/x columns pre-placed by the
                # x staging transpose, forming the K=67 operand for t+1.
                hbn = hb_of[t % 2]
                hbn3 = hbn[:].rearrange("p (g k) -> p g k", k=HBK)
                Xh2n = Xh2_of[(t + 1) % 2]
                hi = lo + SPH - 1
                tcv = tcvs.pop((t, lo))
                for s2 in range(lo, hi + 1):
                    tcs3 = tcv[:].rearrange(
                        "p (g c) -> p g c", c=HID)[
                        :, (s2 - lo) * SUPER:(s2 - lo + 1) * SUPER, :]
                    So2 = Tts.pop((t, s2))[:].rearrange(
                        "p (g c) -> p g c", c=NGATE)[:, :, 128:192]
                    if t < T_STEPS - 1:
                        hb3s = hbn3[:, s2 * SUPER:(s2 + 1) * SUPER, 0:HID]
                        nc.vector.tensor_tensor(hb3s, So2, tcs3, AluOp.mult)
                    else:
                        hf = hf_pool.tile([128, SUPER * HID], F32, tag="hfo")
                        hf3 = hf[:].rearrange("p (g c) -> p g c", c=HID)
                        nc.vector.tensor_tensor(hf3, So2, tcs3, AluOp.mult)
                        hdst = hout[s2 * SCOLS:(s2 + 1) * SCOLS, :].rearrange(
                            "(g e) f -> e g f", e=GROUP)
                        nc.sync.dma_start(hdst, hf3)
                if t < T_STEPS - 1:
                    tw = tr_supers * SUPER * HBK
                    for s2 in range(lo, hi + 1, tr_supers):
                        xdst = Xh2n[
                            :, s2 * SUPER * HBK:s2 * SUPER * HBK + tw
                            ].rearrange("p (b e) -> p b e", e=HBK)
                        nc.sync.dma_start_transpose(
                            xdst,
                            hbn[:, s2 * SUPER * HBK:s2 * SUPER * HBK + tw])

            for t in range(T_STEPS):
                Xh2 = Xh2_of[t % 2]
                hbn = hb_of[t % 2]
                hbn3 = hbn[:].rearrange("p (g k) -> p g k", k=HBK)

                for s in range(NSUPER):
                    if s == 3 and t < T_STEPS - 1:
                        # stage x_{t+1} feature-major, then xbar-transpose
                        # it into hbn's const/x columns (cols 64:80 of each
                        # 128-col group block) ahead of the h-return
                        # transposes; emitted after the chunk1-prev
                        # transposes so those go first on the sync queue
                        xst = xst_of[t % 2]
                        nc.sync.dma_start(xst[0:3, :], xt[t + 1])
                        nc.sync.dma_start_transpose(
                            hbn3[:, :, 64:80], xst[:, :])
                    ph = psum_pool.tile([128, SUPER * NGATE], F32, tag="ps",
                                        name="php")
                    for j in range(SUPER):
                        g = s * SUPER + j
                        nc.tensor.matmul(
                            ph[:, j * NGATE:(j + 1) * NGATE],
                            lhsT=Xh2[0:KROWS,
                                     g * GROUP:(g + 1) * GROUP],
                            rhs=Wg[0:KROWS, :],
                            start=True, stop=True)
                    Tt = t_pool.tile([128, SUPER * NGATE], BF16, tag="T")
                    nc.scalar.activation(
                        Tt[:, :], ph[:, :],
                        mybir.ActivationFunctionType.Tanh)
                    Tts[(t, s)] = Tt

                    T3 = Tt[:].rearrange("p (g c) -> p g c", c=NGATE)
                    Sifo = T3[:, :, 0:192]
                    nc.vector.tensor_scalar(
                        Sifo, Sifo, 1.0, 0.5, AluOp.add, AluOp.mult)
                    Si = T3[:, :, 0:64]
                    Sf = T3[:, :, 64:128]
                    Tg = T3[:, :, 192:256]
                    Cs3 = C[:, s * SUPER * HID:(s + 1) * SUPER * HID
                            ].rearrange("p (g c) -> p g c", c=HID)
                    if t == 0:
                        nc.vector.tensor_tensor(Cs3, Si, Tg, AluOp.mult)
                    else:
                        u = uv_pool.tile([128, SUPER * HID], BF16, tag="u")
                        v = uv_pool.tile([128, SUPER * HID], BF16, tag="v")
                        u3 = u[:].rearrange("p (g c) -> p g c", c=HID)
                        v3 = v[:].rearrange("p (g c) -> p g c", c=HID)
                        nc.vector.tensor_tensor(v3, Si, Tg, AluOp.mult)
                        nc.vector.tensor_tensor(u3, Sf, Cs3, AluOp.mult)
                        nc.vector.tensor_tensor(Cs3, u3, v3, AluOp.add)

                    # software-pipelined chunk tails: tanh_c as soon as the
                    # chunk's cell updates can be ready, hb-mults+transposes
                    # one super later (so they never delay the next supers'
                    # recovery->u->c chain on the DVE queue)
                    if s == SPH + 1:
                        emit_tanh_c(t, 0)
                    if s == SPH + 2:
                        emit_hb_tr(t, 0)
                    if t > 0:
                        if s == 1:
                            emit_tanh_c(t - 1, SPH)
                        if s == 2:
                            emit_hb_tr(t - 1, SPH)
                if t == T_STEPS - 1:
                    emit_tanh_c(t, SPH)
                    emit_hb_tr(t, SPH)
    split_excess_waits(nc)
    return nc


_NC_CACHE = {}


def host_inputs(obs, Wc):
    obs = np.asarray(obs)
    T = obs.shape[0]
    in_maps = []
    for c in range(N_CORES):
        sl = obs[:, c * BC:(c + 1) * BC, :]          # [T, BC, 2]
        xT = np.empty((T, 3, BC), np.float32)
        xT[:, 0, :] = 1.0
        xT[:, 1:3, :] = sl.transpose(0, 2, 1)
        in_maps.append({"xt": xT.astype(ml_dtypes.bfloat16), "wg": Wc})
    return in_maps


def host_gather(res):
    h = np.concatenate([r["hout"] for r in res.results], axis=0)
    return h[None].astype(np.float32)


def kernel(obs_traj, W_emb, b_emb, W_ih, W_hh, b_ih, b_hh):
    from concourse.bass_utils import run_bass_kernel_spmd

    Wc = host_weights(
        np.asarray(W_emb, dtype=np.float32),
        np.asarray(b_emb, dtype=np.float32),
        np.asarray(W_ih, dtype=np.float32),
        np.asarray(W_hh, dtype=np.float32),
        np.asarray(b_ih, dtype=np.float32),
        np.asarray(b_hh, dtype=np.float32))
    in_maps = host_inputs(obs_traj, Wc)
    if "nc" not in _NC_CACHE:
        _NC_CACHE["nc"] = build_nc()
    res = run_bass_kernel_spmd(_NC_CACHE["nc"], in_maps,
                               core_ids=list(range(N_CORES)))
    return host_gather(res)
